# revision 13
# baseline (speedup 1.0000x reference)
"""GCMC (NGCF-style) forward on 8 Trainium2 NeuronCores — v2.

Replaces SWDGE dma_gather (7ns/descriptor on GpSimd was the v1 wall) with
SBUF-resident tables + gpsimd ap_gather:

- Nodes split into 4 column-groups of 37632. Each layer runs 2 passes; pass t
  holds groups {2t, 2t+1} in SBUF as an f32 table [128, 2*18816]: partition
  p = (dim p%64, group-parity p//64), free = (range ri, local node).
- ap_gather (d=1, f32) pulls per-edge source values as G^T tiles: Q7 cores
  0-3 serve even-group edge streams, 4-7 odd-group, in one call.
- PE transpose turns each [128,128] G^T slice into a pair tile
  [128 edges, 64 even-dims | 64 odd-dims]; batched ACT copies convert
  PSUM->SBUF bf16.
- One-hot M matrices ([128 edges, 128 rows] bf16) are built on DVE with
  fused is_equal*val tensor_scalar ops, then bf16 matmuls accumulate
  side^T per 128-row destination block in PSUM.
- Dense 64x64 layers run per 3-block window in bf16; outputs are written
  both as transposed slabs (next layer's table, layer 0 only) and dense
  rows (BPR gathers), AllGathered in bf16.
"""
import numpy as np
import ml_dtypes

import concourse.bass as bass
import concourse.bacc as bacc
import concourse.mybir as mybir
import concourse.tile as tile
from concourse.bass_utils import run_bass_kernel_spmd
from concourse import library_config

U, I, D = 100000, 50000, 64
N = U + I
E = 3_000_000
B = 8192
NEG_SLOPE = 0.2
REG_LAMBDA = 1e-4

NCORES = 8
P = 128
NBLK = 147
RPC = P * NBLK                # 18816 rows per core
NP_ = NCORES * RPC            # 150528 padded node count
GRP = 37632                   # nodes per column-group (4 groups)
RSZ = 18816                   # nodes per range (2 ranges per group)
W = 3                         # blocks per window
NW = -(-NBLK // W)            # 49 windows
BPC = B // NCORES
BJ = BPC // P

F32 = mybir.dt.float32
BF16 = mybir.dt.bfloat16
I32 = mybir.dt.int32
I16 = mybir.dt.int16
AF = mybir.ActivationFunctionType
ALU = mybir.AluOpType
BF = ml_dtypes.bfloat16


def prep(inputs):
    rows = np.asarray(inputs["rows"], np.int64)
    cols = np.asarray(inputs["cols"], np.int64)
    vals = np.asarray(inputs["vals"], np.float32)

    core = rows // RPC
    b_loc = (rows % RPC) >> 7
    brow = (rows & 127).astype(np.float32)
    grp = cols // GRP                  # 0..3
    ri = (cols % GRP) // RSZ           # 0..1
    loc = (cols % RSZ).astype(np.int16)

    # bin = (block, grp, ri) -> chunk counts, maxed over cores
    NBIN = NBLK * 4 * 2
    key_all = (b_loc * 8 + grp * 2 + ri).astype(np.int64)
    cnts = np.zeros((NCORES, NBIN), np.int64)
    per_core = []
    for k in range(NCORES):
        m = core == k
        cnts[k] = np.bincount(key_all[m], minlength=NBIN)
        per_core.append((key_all[m], brow[m], loc[m], vals[m]))
    C = -(-np.max(cnts, axis=0) // P)       # chunks per bin

    windows = [list(range(s, min(s + W, NBLK))) for s in range(0, NBLK, W)]

    # schedule: for (t, w, ri): npair slots; chunk lists per parity
    # chunk-slot global counter s; chunk (b,g,ri) occupies slots
    # [start[bin], start[bin]+C[bin])  within its (t,w,ri) region.
    sched = []          # per (t, w): dict(ri -> (s0, npair, ev_list, od_list))
    slot_of_bin = np.zeros(NBIN, np.int64)  # first slot of bin's chunks
    NSLOT = 0
    for t in range(2):
        for wi, blocks in enumerate(windows):
            ent = {}
            for r in range(2):
                ev, od = [], []
                for b in blocks:
                    bin_e = b * 8 + (2 * t) * 2 + r
                    bin_o = b * 8 + (2 * t + 1) * 2 + r
                    for c in range(C[bin_e]):
                        ev.append((b, bin_e, c))
                    for c in range(C[bin_o]):
                        od.append((b, bin_o, c))
                npair = max(len(ev), len(od))
                s0 = NSLOT
                for j, (b, bn, c) in enumerate(ev):
                    if c == 0:
                        slot_of_bin[bn] = s0 + j
                for j, (b, bn, c) in enumerate(od):
                    if c == 0:
                        slot_of_bin[bn] = s0 + j
                NSLOT += npair
                ent[r] = (s0, npair, ev, od)
            sched.append(ent)

    # per-core streams: place each edge at (slot, lane) in its parity stream
    idx_maps, meta_maps = [], []
    for k in range(NCORES):
        key, rr_, lc_, vv_ = per_core[k]
        order = np.argsort(key, kind="stable")
        key_s, rr_s, lc_s, vv_s = key[order], rr_[order], lc_[order], vv_[order]
        gstart = np.zeros(NBIN + 1, np.int64)
        np.cumsum(np.bincount(key_s, minlength=NBIN), out=gstart[1:])
        within = np.arange(len(key_s)) - gstart[key_s]
        slot = slot_of_bin[key_s] + (within >> 7)
        lane = within & 127

        # idx: [128, NSLOT*8] int16; even stream on partitions 0..63 (4
        # cores), odd on 64..127. Within a slot, lane j of the stream is
        # packed at [16c + j%16, slot*8 + j//16].
        rr_e = np.zeros((NSLOT, P), np.float32)
        vv_e = np.zeros((NSLOT, P), np.float32)
        rr_o = np.zeros((NSLOT, P), np.float32)
        vv_o = np.zeros((NSLOT, P), np.float32)
        par = (key_s // 2) & 1
        ev_m = par == 0
        od_m = par == 1
        rr_e[slot[ev_m], lane[ev_m]] = rr_s[ev_m]
        vv_e[slot[ev_m], lane[ev_m]] = vv_s[ev_m]
        rr_o[slot[od_m], lane[od_m]] = rr_s[od_m]
        vv_o[slot[od_m], lane[od_m]] = vv_s[od_m]
        # but even/odd streams gather DIFFERENT idx on different partitions:
        idx_e = np.zeros((NSLOT, P), np.int16)
        idx_o = np.zeros((NSLOT, P), np.int16)
        idx_e[slot[ev_m], lane[ev_m]] = lc_s[ev_m]
        idx_o[slot[od_m], lane[od_m]] = lc_s[od_m]

        idx_pack = np.zeros((P, NSLOT * 8), np.int16)
        je = idx_e.reshape(NSLOT * 8, 16)   # [slotcol, 16lane]
        jo = idx_o.reshape(NSLOT * 8, 16)
        for c4 in range(4):
            idx_pack[16 * c4:16 * c4 + 16, :] = je.T
            idx_pack[64 + 16 * c4:64 + 16 * c4 + 16, :] = jo.T

        meta = np.concatenate([rr_e.T, vv_e.T, rr_o.T, vv_o.T], axis=1)
        idx_maps.append(np.ascontiguousarray(idx_pack))
        meta_maps.append(np.ascontiguousarray(meta))

    # tables / weights
    ego0 = np.concatenate([np.asarray(inputs["user_emb"], np.float32),
                           np.asarray(inputs["item_emb"], np.float32)],
                          axis=0)
    ego0_pad = np.zeros((NP_, D), np.float32)
    ego0_pad[:N] = ego0
    ego0_dense = ego0_pad.astype(BF)
    tab0 = np.ascontiguousarray(
        ego0_pad.reshape(NCORES, RPC, D).transpose(0, 2, 1).astype(BF))

    fc = np.concatenate([
        np.asarray(inputs["W_gcn0"], np.float32),
        np.asarray(inputs["W_mlp0"], np.float32),
        np.asarray(inputs["W_gcn1"], np.float32),
        np.asarray(inputs["W_mlp1"], np.float32),
        np.eye(D, dtype=np.float32),
    ], axis=1).astype(BF)
    fb = np.concatenate([
        0.8 * np.asarray(inputs["b_gcn0"], np.float32).T,
        0.2 * np.asarray(inputs["b_gcn0"], np.float32).T,
        np.asarray(inputs["b_mlp0"], np.float32).T,
        0.8 * np.asarray(inputs["b_gcn1"], np.float32).T,
        0.2 * np.asarray(inputs["b_gcn1"], np.float32).T,
        np.asarray(inputs["b_mlp1"], np.float32).T,
    ], axis=1)

    user = np.asarray(inputs["user"], np.int64)
    pos_i = np.asarray(inputs["positive"], np.int64) + U
    neg_i = np.asarray(inputs["negative"], np.int64) + U
    in_maps = []
    for k in range(NCORES):
        s0 = k * BPC
        bidx = np.concatenate([
            user[s0:s0 + BPC].reshape(P, BJ),
            pos_i[s0:s0 + BPC].reshape(P, BJ),
            neg_i[s0:s0 + BPC].reshape(P, BJ),
        ], axis=1).astype(np.int32)
        in_maps.append(dict(
            idx_all=idx_maps[k], meta=meta_maps[k],
            bidx=np.ascontiguousarray(bidx),
            ego0_dense=ego0_dense, tab0=tab0,
            fconst=np.ascontiguousarray(fc),
            fbias=np.ascontiguousarray(fb),
            identf=np.eye(P, dtype=np.float32),
            iotab=np.tile(np.arange(P, dtype=np.float32), (P, 1)).astype(BF),
        ))
    return dict(sched=sched, NSLOT=NSLOT, windows=windows), in_maps


def build(hs):
    sched, NSLOT, windows = hs["sched"], hs["NSLOT"], hs["windows"]
    maxLw = max(ent[0][1] + ent[1][1] for ent in sched)

    nc = bacc.Bacc()
    idx_all = nc.dram_tensor("idx_all", [P, NSLOT * 8], I16,
                             kind="ExternalInput")
    meta_d = nc.dram_tensor("meta", [P, 4, NSLOT], F32,
                            kind="ExternalInput")
    iotab_d = nc.dram_tensor("iotab", [P, P], BF16, kind="ExternalInput")
    bidx_d = nc.dram_tensor("bidx", [P, 3 * BJ], I32, kind="ExternalInput")
    ego0_dense = nc.dram_tensor("ego0_dense", [NP_, D], BF16,
                                kind="ExternalInput")
    tab0 = nc.dram_tensor("tab0", [NCORES, D, RPC], BF16,
                          kind="ExternalInput")
    fconst = nc.dram_tensor("fconst", [D, 4 * D + D], BF16,
                            kind="ExternalInput")
    fbias = nc.dram_tensor("fbias", [D, 6], F32, kind="ExternalInput")
    identf_d = nc.dram_tensor("identf", [P, P], F32, kind="ExternalInput")
    out_ext = nc.dram_tensor("out", [1, 2], F32, kind="ExternalOutput")

    side0_d = nc.dram_tensor("side0", [D, RPC], F32)
    egoT1_blk = nc.dram_tensor("egoT1_blk", [D, RPC], BF16)
    egoT1_ag = nc.dram_tensor("egoT1_ag", [NCORES, D, RPC], BF16,
                              addr_space="Shared")
    dense_blk = [nc.dram_tensor(f"dense{l}_blk", [RPC, D], BF16)
                 for l in (1, 2)]
    dense_ag = [nc.dram_tensor(f"dense{l}_ag", [NP_, D], BF16,
                               addr_space="Shared") for l in (1, 2)]
    ar_in = nc.dram_tensor("ar_in", [1, 8], F32)
    ar_out = nc.dram_tensor("ar_out", [1, 8], F32, addr_space="Shared")
    RGRP = [list(range(NCORES))]

    with tile.TileContext(nc) as tc:
        nc.gpsimd.load_library(library_config.ap_gather)
        with (
            tc.tile_pool(name="const", bufs=1) as cp,
            tc.tile_pool(name="sp", bufs=2) as sp,
            tc.tile_pool(name="gp", bufs=2) as gp,
            tc.tile_pool(name="pp", bufs=2, space="PSUM") as pp,
        ):
            fc_sb = cp.tile([D, 4 * D + D], BF16)
            nc.sync.dma_start(fc_sb[:], fconst[:])
            fb_sb = cp.tile([D, 6], F32)
            nc.sync.dma_start(fb_sb[:], fbias[:])
            bidx_sb = cp.tile([P, 3 * BJ], I32)
            nc.sync.dma_start(bidx_sb[:], bidx_d[:])
            iota_sb = cp.tile([P, P], BF16)
            nc.sync.dma_start(iota_sb[:], iotab_d[:])
            identf = cp.tile([P, P], F32)
            nc.sync.dma_start(identf[:], identf_d[:])

            w_g = [fc_sb[:, 0:D], fc_sb[:, 2 * D:3 * D]]
            w_m = [fc_sb[:, D:2 * D], fc_sb[:, 3 * D:4 * D]]
            identb = fc_sb[:, 4 * D:5 * D]
            bg08 = [fb_sb[:, 0:1], fb_sb[:, 3:4]]
            bg02 = [fb_sb[:, 1:2], fb_sb[:, 4:5]]
            bm = [fb_sb[:, 2:3], fb_sb[:, 5:6]]

            tabsb = cp.tile([P, 2, RSZ, 1], F32)

            # ---- BPR gather + stats ----------------------------------
            ss, dp, dn = {}, {}, {}

            def bpr_layer(l, table):
                gbs = []
                for role in range(3):
                    g = sp.tile([P, BJ, D], BF16, tag="gb", bufs=3)
                    for j in range(BJ):
                        nc.gpsimd.indirect_dma_start(
                            out=g[:, j, :], out_offset=None, in_=table[:],
                            in_offset=bass.IndirectOffsetOnAxis(
                                ap=bidx_sb[:, role * BJ + j:role * BJ + j + 1],
                                axis=0))
                    gbs.append(g)
                for role in range(3):
                    s = cp.tile([P, BJ], F32, name=f"ss{l}_{role}")
                    for j in range(BJ):
                        sq = sp.tile([P, D], F32, tag="sq", bufs=2)
                        nc.scalar.activation(sq[:], gbs[role][:, j, :],
                                             AF.Square,
                                             accum_out=s[:, j:j + 1])
                    ss[(l, role)] = s
                for role, dst in ((1, dp), (2, dn)):
                    d_ = cp.tile([P, BJ], F32, name=f"d{l}_{role}")
                    for j in range(BJ):
                        m = sp.tile([P, D], F32, tag="dm", bufs=2)
                        nc.vector.tensor_tensor(m[:], gbs[0][:, j, :],
                                                gbs[role][:, j, :], ALU.mult)
                        nc.vector.tensor_reduce(d_[:, j:j + 1], m[:],
                                                mybir.AxisListType.X, ALU.add)
                    dst[l] = d_

            bpr_layer(0, ego0_dense)

            # ---- propagation layers (software-pipelined windows) -----
            # Per-parity real-chunk capacity of the per-window M tiles.
            MMAX = max(max(len(e[0][2]) + len(e[1][2]),
                           len(e[0][3]) + len(e[1][3])) for e in sched)

            def emit_prefetch(t, wi):
                """Issue window (t, wi)'s input DMAs, gathers and M builds
                ahead of the previous window's compute so Pool/DVE overlap
                PE/ACT."""
                ent = sched[t * NW + wi]
                s0w = ent[0][0]
                Lw = ent[0][1] + ent[1][1]
                idx_w = sp.tile([P, maxLw * 8], I16, tag="idxw")
                nc.sync.dma_start(idx_w[:, :Lw * 8],
                                  idx_all[:, s0w * 8:(s0w + Lw) * 8])
                met_w = sp.tile([P, 4, maxLw], F32, tag="metw")
                nc.sync.dma_start(met_w[:, :, :Lw],
                                  meta_d[:, :, s0w:s0w + Lw])
                GT = gp.tile([P, maxLw * P, 1], F32, tag="GT")
                for r in range(2):
                    sr, npair, _, _ = ent[r]
                    if npair == 0:
                        continue
                    off = sr - s0w
                    nc.gpsimd.ap_gather(
                        out_ap=GT[:, off * P:(off + npair) * P, :],
                        in_ap=tabsb[:, r:r + 1, :, :].rearrange(
                            "p a b c -> p (a b) c"),
                        idxs_ap=idx_w[:, off * 8:(off + npair) * 8],
                        channels=P, num_elems=RSZ, d=1,
                        num_idxs=npair * P)
                mts = []
                for par in range(2):
                    mt = sp.tile([P, MMAX, P], BF16, tag=f"M{par}", bufs=2)
                    q = 0
                    for r in range(2):
                        sr, npair, ev, od = ent[r]
                        off = sr - s0w
                        for j, (b, bn, c) in enumerate(ev if par == 0
                                                       else od):
                            sl = off + j
                            nc.vector.tensor_scalar(
                                mt[:, q, :], iota_sb[:],
                                met_w[:, 2 * par, sl:sl + 1],
                                met_w[:, 2 * par + 1, sl:sl + 1],
                                ALU.is_equal, ALU.mult)
                            q += 1
                    mts.append(mt)
                return dict(ent=ent, s0w=s0w, Lw=Lw, GT=GT, mts=mts)

            def emit_compute(l, t, wi, pf):
                blocks = windows[wi]
                ent, s0w, Lw, GT, mts = (pf["ent"], pf["s0w"], pf["Lw"],
                                         pf["GT"], pf["mts"])
                # pair tiles: transpose 4 slots at a time, convert to bf16
                Gc = {}
                for c0 in range(0, Lw, 4):
                    cn = min(4, Lw - c0)
                    ps = pp.tile([P, 4, P], F32, tag="tp")
                    for j in range(cn):
                        nc.tensor.transpose(
                            ps[:, j, :],
                            GT[:, (c0 + j) * P:(c0 + j + 1) * P, 0],
                            identf[:])
                    gc = sp.tile([P, 4, P], BF16, tag="gc", bufs=3)
                    nc.scalar.activation(
                        gc[:, :cn, :].rearrange("p a b -> p (a b)"),
                        ps[:, :cn, :].rearrange("p a b -> p (a b)"),
                        AF.Copy)
                    Gc[c0 // 4] = gc

                psd = pp.tile([D, W, P], F32, tag="side")
                first = {b: True for b in blocks}
                nmm = {b: 0 for b in blocks}
                for r in range(2):
                    _, _, ev, od = ent[r]
                    for lst in (ev, od):
                        for (b, bn, c) in lst:
                            nmm[b] += 1
                qq = [0, 0]
                for r in range(2):
                    sr, npair, ev, od = ent[r]
                    off = sr - s0w
                    for par, lst in ((0, ev), (1, od)):
                        for j, (b, bn, c) in enumerate(lst):
                            sl = off + j
                            bw = blocks.index(b)
                            nmm[b] -= 1
                            nc.tensor.matmul(
                                psd[:, bw, :],
                                lhsT=Gc[sl // 4][:, sl % 4,
                                                 64 * par:64 * par + 64],
                                rhs=mts[par][:, qq[par], :],
                                start=first[b], stop=(nmm[b] == 0))
                            first[b] = False
                            qq[par] += 1

                nb = len(blocks)
                if t == 0:
                    sp0 = sp.tile([D, W * P], F32, tag="sp0", bufs=1)
                    nc.scalar.activation(
                        sp0[:, :nb * P],
                        psd[:, :nb, :].rearrange("p a b -> p (a b)"),
                        AF.Copy)
                    nc.sync.dma_start(
                        side0_d[:, wi * W * P:wi * W * P + nb * P],
                        sp0[:, :nb * P])
                else:
                    s0sb = sp.tile([D, W * P], F32, tag="s0l", bufs=1)
                    nc.sync.dma_start(
                        s0sb[:, :nb * P],
                        side0_d[:, wi * W * P:wi * W * P + nb * P])
                    sideT = sp.tile([D, W * P], BF16, tag="sideT")
                    nc.vector.tensor_tensor(
                        sideT[:, :nb * P],
                        psd[:, :nb, :].rearrange("p a b -> p (a b)"),
                        s0sb[:, :nb * P], ALU.add)
                    p1 = pp.tile([D, W * P], F32, tag="dns")
                    nc.tensor.matmul(p1[:, :nb * P], lhsT=w_g[l],
                                     rhs=sideT[:, :nb * P],
                                     start=True, stop=True)
                    relu8 = sp.tile([D, W * P], BF16, tag="r8")
                    nc.scalar.activation(relu8[:, :nb * P],
                                         p1[:, :nb * P], AF.Relu,
                                         bias=bg08[l], scale=0.8)
                    uu = sp.tile([D, W * P], BF16, tag="uu")
                    nc.vector.tensor_scalar(uu[:, :nb * P],
                                            p1[:, :nb * P], 0.2,
                                            bg02[l], ALU.mult, ALU.add)
                    gcnT = sp.tile([D, W * P], BF16, tag="gcnT")
                    nc.vector.tensor_tensor(gcnT[:, :nb * P],
                                            uu[:, :nb * P],
                                            relu8[:, :nb * P], ALU.add)
                    p2 = pp.tile([D, W * P], F32, tag="dns")
                    nc.tensor.matmul(p2[:, :nb * P], lhsT=w_m[l],
                                     rhs=gcnT[:, :nb * P],
                                     start=True, stop=True)
                    egoT = sp.tile([D, W * P], BF16, tag="egoT")
                    nc.scalar.activation(egoT[:, :nb * P],
                                         p2[:, :nb * P], AF.Identity,
                                         bias=bm[l])
                    if l == 0:
                        nc.sync.dma_start(
                            egoT1_blk[:, wi * W * P:wi * W * P + nb * P],
                            egoT[:, :nb * P])
                    p3 = pp.tile([P, W, D], BF16, tag="p3", bufs=1)
                    for j in range(nb):
                        nc.tensor.transpose(
                            p3[:, j, :], egoT[:, j * P:(j + 1) * P],
                            fc_sb[:, 4 * D:5 * D])
                    nat = sp.tile([P, W, D], BF16, tag="nat", bufs=1)
                    nc.scalar.activation(
                        nat[:, :nb, :].rearrange("p a b -> p (a b)"),
                        p3[:, :nb, :].rearrange("p a b -> p (a b)"),
                        AF.Copy)
                    nc.sync.dma_start(
                        dense_blk[l][wi * W * P:wi * W * P + nb * P,
                                     :].rearrange("(a p) d -> p a d", p=P),
                        nat[:, :nb, :])

            for l in range(2):
                for t in range(2):
                    for m in range(4):
                        slab = 4 * t + m
                        src = tab0[slab] if l == 0 else egoT1_ag[slab]
                        nc.gpsimd.dma_start(
                            tabsb[64 * (m // 2):64 * (m // 2) + 64,
                                  m % 2:m % 2 + 1, :, :].rearrange(
                                "p a b c -> p (a b c)"),
                            src[:, :])
                    pf = emit_prefetch(t, 0)
                    for wi in range(NW):
                        nxt = (emit_prefetch(t, wi + 1)
                               if wi + 1 < NW else None)
                        emit_compute(l, t, wi, pf)
                        pf = nxt

                if l == 0:
                    nc.gpsimd.collective_compute(
                        "AllGather", ALU.bypass, replica_groups=RGRP,
                        ins=[egoT1_blk[:]], outs=[egoT1_ag[:]])
                nc.gpsimd.collective_compute(
                    "AllGather", ALU.bypass, replica_groups=RGRP,
                    ins=[dense_blk[l][:]], outs=[dense_ag[l][:]])
                bpr_layer(l + 1, dense_ag[l])

            # ---- final combine --------------------------------------
            def norm_term(d_, su, so):
                tt = sp.tile([P, BJ], F32, tag="nt", bufs=6)
                nc.vector.tensor_tensor(tt[:], su[:], so[:], ALU.mult)
                t2 = sp.tile([P, BJ], F32, tag="nt", bufs=6)
                nc.scalar.activation(t2[:], tt[:], AF.Sqrt)
                t3 = sp.tile([P, BJ], F32, tag="nt", bufs=6)
                nc.vector.reciprocal(t3[:], t2[:])
                t4 = sp.tile([P, BJ], F32, tag="nt", bufs=6)
                nc.vector.tensor_tensor(t4[:], d_[:], t3[:], ALU.mult)
                return t4

            pos_s = cp.tile([P, BJ], F32)
            nc.vector.tensor_tensor(pos_s[:], dp[0][:],
                                    norm_term(dp[1], ss[(1, 0)],
                                              ss[(1, 1)])[:], ALU.add)
            nc.vector.tensor_tensor(pos_s[:], pos_s[:],
                                    norm_term(dp[2], ss[(2, 0)],
                                              ss[(2, 1)])[:], ALU.add)
            neg_s = cp.tile([P, BJ], F32)
            nc.vector.tensor_tensor(neg_s[:], dn[0][:],
                                    norm_term(dn[1], ss[(1, 0)],
                                              ss[(1, 2)])[:], ALU.add)
            nc.vector.tensor_tensor(neg_s[:], neg_s[:],
                                    norm_term(dn[2], ss[(2, 0)],
                                              ss[(2, 2)])[:], ALU.add)
            xdiff = cp.tile([P, BJ], F32)
            nc.vector.tensor_tensor(xdiff[:], neg_s[:], pos_s[:],
                                    ALU.subtract)
            ex = cp.tile([P, BJ], F32)
            nc.scalar.activation(ex[:], xdiff[:], AF.Exp)
            sp_ = cp.tile([P, BJ], F32)
            nc.scalar.activation(sp_[:], ex[:], AF.Ln, bias=1.0)

            reg_row = cp.tile([P, BJ], F32)
            nc.vector.tensor_tensor(reg_row[:], ss[(0, 0)][:],
                                    ss[(0, 1)][:], ALU.add)
            nc.vector.tensor_tensor(reg_row[:], reg_row[:], ss[(0, 2)][:],
                                    ALU.add)

            sc = cp.tile([P, 2], F32)
            srow = cp.tile([P, 1], F32)
            nc.vector.tensor_reduce(srow[:], sp_[:], mybir.AxisListType.X,
                                    ALU.add)
            nc.scalar.activation(sc[:, 0:1], srow[:], AF.Copy, scale=1.0 / B)
            rrow = cp.tile([P, 1], F32)
            nc.vector.tensor_reduce(rrow[:], reg_row[:],
                                    mybir.AxisListType.X, ALU.add)
            nc.scalar.activation(sc[:, 1:2], rrow[:], AF.Copy,
                                 scale=REG_LAMBDA * 0.5 / B)
            ones = cp.tile([P, 1], F32)
            nc.vector.memset(ones[:], 1.0)
            tot = pp.tile([1, 2], F32, tag="tot", bufs=1)
            nc.tensor.matmul(tot[:], lhsT=ones[:], rhs=sc[:], start=True,
                             stop=True)
            ar_sb = cp.tile([1, 8], F32)
            nc.vector.memset(ar_sb[:], 0.0)
            nc.scalar.copy(ar_sb[:, 0:2], tot[:])
            nc.sync.dma_start(ar_in[:], ar_sb[:])
            nc.gpsimd.collective_compute(
                "AllReduce", ALU.add, replica_groups=RGRP,
                ins=[ar_in[:]], outs=[ar_out[:]])
            nc.sync.dma_start(out_ext[:], ar_out[:1, 0:2])
    nc.compile()
    return nc


def run(inputs, trace=False, trace_cores=None):
    inputs = {k: np.asarray(v) for k, v in inputs.items()}
    hs, in_maps = prep(inputs)
    nc = build(hs)
    kw = {}
    if trace:
        kw = dict(trace=True, trace_cores=trace_cores or [0])
    res = run_bass_kernel_spmd(nc, in_maps, list(range(NCORES)), **kw)
    out = res.results[0]["out"].reshape(2).astype(np.float32)
    return out, res


def kernel(**inputs):
    out, _ = run(inputs)
    return out


# revision 16
# speedup vs baseline: 1.0009x; 1.0009x over previous
"""GCMC (NGCF-style) forward on 8 Trainium2 NeuronCores.

Sharding: edges are partitioned by destination-row range (18816 rows/core).
Each core computes its row-block of both propagation layers via one-hot
matmul segment-sum (PSUM-accumulated), cores AllGather the updated node
table between layers, and the BPR batch is data-parallel (1024 slots/core)
with a final 2-scalar AllReduce.

Node tables are stored in a host-side permuted ("kpb") layout
  table_row(node r) = (core(r)*128 + (r%18816)%128) * 147 + (r%18816)//128
so each core's computed block is a single contiguous DMA and AllGather
concatenation reproduces the layout. All gather indices are pre-permuted on
the host accordingly.
"""
import numpy as np

import concourse.bass as bass
import concourse.bacc as bacc
import concourse.mybir as mybir
import concourse.tile as tile
from concourse.bass_utils import run_bass_kernel_spmd
from concourse.library_config import mlp as _mlp_lib

U, I, D = 100000, 50000, 64
N = U + I
E = 3_000_000
B = 8192
NEG_SLOPE = 0.2
REG_LAMBDA = 1e-4

NCORES = 8
P = 128
NBLK = 147                    # 128-row blocks per core
RPC = P * NBLK                # 18816 rows per core
NP_ = NCORES * RPC            # 150528 padded node count
NRANGE = 5                    # int16 index ranges of 32768 table rows
RANGE_ROWS = 32768
W_BLOCKS = 6                  # blocks per gather window
BPC = B // NCORES             # 1024 BPR slots per core
BJ = BPC // P                 # 8 slot groups per core

F32 = mybir.dt.float32
I32 = mybir.dt.int32
I16 = mybir.dt.int16
AF = mybir.ActivationFunctionType
ALU = mybir.AluOpType


def _perm(nodes):
    """node id -> row in the kpb-permuted table."""
    k = nodes // RPC
    loc = nodes % RPC
    return (k * P + loc % P) * NBLK + loc // P


def _pack_idx16(flat):
    """flat int16 idx list (len % 128 == 0) -> [128, len/16] dma_gather layout."""
    L = len(flat)
    a = flat.reshape(L // 16, 16).T          # idx i at [i%16, i//16]
    return np.tile(a, (NCORES, 1)).copy()    # replicate to 128 partitions


def prep(inputs):
    """Host-side sharding. Returns (sched, in_maps_extra) where sched drives
    program construction and in_maps_extra has per-core input arrays."""
    rows = np.asarray(inputs["rows"], np.int64)
    cols = np.asarray(inputs["cols"], np.int64)
    vals = np.asarray(inputs["vals"], np.float32)

    cperm = _perm(cols)
    ridx = cperm >> 15                 # range id 0..4
    lidx = (cperm & 32767).astype(np.int16)
    core = rows // RPC

    per_core = []
    cnts = np.zeros((NCORES, NBLK * NRANGE), np.int64)
    for k in range(NCORES):
        m = core == k
        r_loc = rows[m] - k * RPC
        bb = r_loc >> 7
        rr = (r_loc & 127).astype(np.float32)
        key = bb * NRANGE + ridx[m]
        cnts[k] = np.bincount(key, minlength=NBLK * NRANGE)
        per_core.append((key, rr, lidx[m], vals[m]))

    C = -(-cnts.max(axis=0) // P)            # [NBLK*NRANGE] chunks per (b, ri)

    # windows of W_BLOCKS blocks; chunk columns ordered (window, range, block, c)
    windows = [list(range(s, min(s + W_BLOCKS, NBLK)))
               for s in range(0, NBLK, W_BLOCKS)]
    col_start = np.zeros(NBLK * NRANGE, np.int64)   # first chunk col of (b, ri)
    win_info = []   # per window: (wstart, nw, [(ri, col_off_in_window, nchunks)])
    block_cols = [[] for _ in range(NBLK)]          # per block: chunk cols in order
    pos = 0
    for blocks in windows:
        wstart = pos
        parts = []
        for ri in range(NRANGE):
            n_r = 0
            for b in blocks:
                col_start[b * NRANGE + ri] = pos
                block_cols[b].extend(range(pos, pos + C[b * NRANGE + ri]))
                pos += C[b * NRANGE + ri]
                n_r += C[b * NRANGE + ri]
            parts.append((ri, wstart, n_r))
        # fix per-range offsets inside the window (they are cumulative)
        off = 0
        parts2 = []
        for ri, _, n_r in parts:
            parts2.append((ri, off, n_r))
            off += n_r
        win_info.append((wstart, pos - wstart, parts2))
    nchunk = pos

    chunk_ri = np.zeros(nchunk, np.int64)
    for bri in range(NBLK * NRANGE):
        cs, cn = col_start[bri], C[bri]
        chunk_ri[cs:cs + cn] = bri % NRANGE
    sched = dict(nchunk=nchunk, windows=windows, win_info=win_info,
                 block_cols=block_cols, chunk_ri=chunk_ri, C=C)

    # per-core edge arrays
    iota = np.tile(np.arange(P, dtype=np.float32), (P, 1))
    in_maps = []
    ego0 = np.concatenate([np.asarray(inputs["user_emb"], np.float32),
                           np.asarray(inputs["item_emb"], np.float32)], axis=0)
    ego0_pad = np.zeros((NP_, D), np.float32)
    ego0_pad[:N] = ego0
    t_of_node = _perm(np.arange(NP_))
    ego0_perm = np.zeros((NP_, D), np.float32)
    ego0_perm[t_of_node] = ego0_pad

    user = np.asarray(inputs["user"], np.int64)
    pos_i = np.asarray(inputs["positive"], np.int64)
    neg_i = np.asarray(inputs["negative"], np.int64)
    uP = _perm(user).astype(np.int32)
    pP = _perm(U + pos_i).astype(np.int32)
    nP = _perm(U + neg_i).astype(np.int32)

    # fconst: [w_gcn0 | w_mlp0 | w_gcn1 | w_mlp1 | 6 bias cols | ident]
    fc = np.concatenate([
        np.asarray(inputs["W_gcn0"], np.float32),
        np.asarray(inputs["W_mlp0"], np.float32),
        np.asarray(inputs["W_gcn1"], np.float32),
        np.asarray(inputs["W_mlp1"], np.float32),
        0.8 * np.asarray(inputs["b_gcn0"], np.float32).T,
        0.2 * np.asarray(inputs["b_gcn0"], np.float32).T,
        np.asarray(inputs["b_mlp0"], np.float32).T,
        0.8 * np.asarray(inputs["b_gcn1"], np.float32).T,
        0.2 * np.asarray(inputs["b_gcn1"], np.float32).T,
        np.asarray(inputs["b_mlp1"], np.float32).T,
        np.eye(D, dtype=np.float32),
    ], axis=1)

    for k in range(NCORES):
        key, rr, li, vv = per_core[k]
        order = np.argsort(key, kind="stable")
        key_s, rr_s, li_s, vv_s = key[order], rr[order], li[order], vv[order]
        gstart = np.zeros(NBLK * NRANGE + 1, np.int64)
        np.cumsum(np.bincount(key_s, minlength=NBLK * NRANGE), out=gstart[1:])
        within = np.arange(len(key_s)) - gstart[key_s]
        chunkcol = col_start[key_s] + within // P
        lane = within % P

        rows_arr = np.zeros((nchunk, P), np.float32)
        vals_arr = np.zeros((nchunk, P), np.float32)
        idx_arr = np.zeros((nchunk, P), np.int16)
        rows_arr[chunkcol, lane] = rr_s
        vals_arr[chunkcol, lane] = vv_s
        idx_arr[chunkcol, lane] = li_s

        meta = np.concatenate([rows_arr.T, vals_arr.T, iota], axis=1)
        idx_all = _pack_idx16(idx_arr.reshape(-1))   # [128, nchunk*8]

        s0 = k * BPC
        bidx = np.concatenate([
            uP[s0:s0 + BPC].reshape(P, BJ),
            pP[s0:s0 + BPC].reshape(P, BJ),
            nP[s0:s0 + BPC].reshape(P, BJ),
        ], axis=1)

        in_maps.append(dict(
            ego0=ego0_perm, meta=np.ascontiguousarray(meta),
            idx_all=np.ascontiguousarray(idx_all),
            fconst=np.ascontiguousarray(fc), bidx=np.ascontiguousarray(bidx),
        ))
    return sched, in_maps


def build(sched):
    nchunk = sched["nchunk"]
    win_info = sched["win_info"]
    windows = sched["windows"]
    block_cols = sched["block_cols"]
    max_nw = max(nw for _, nw, _ in win_info)

    nc = bacc.Bacc()
    ego0 = nc.dram_tensor("ego0", [NP_, D], F32, kind="ExternalInput")
    meta = nc.dram_tensor("meta", [P, 2 * nchunk + P], F32, kind="ExternalInput")
    idx_all = nc.dram_tensor("idx_all", [P, nchunk * 8], I16, kind="ExternalInput")
    fconst = nc.dram_tensor("fconst", [D, 4 * D + 6 + D], F32, kind="ExternalInput")
    bidx = nc.dram_tensor("bidx", [P, 3 * BJ], I32, kind="ExternalInput")
    out_ext = nc.dram_tensor("out", [1, 2], F32, kind="ExternalOutput")

    ego_blk = [nc.dram_tensor(f"ego{l}_blk", [RPC, D], F32) for l in (1, 2)]
    ego_full = [nc.dram_tensor(f"ego{l}_full", [NP_, D], F32, addr_space="Shared")
                for l in (1, 2)]
    ar_in = nc.dram_tensor("ar_in", [1, 8], F32)
    ar_out = nc.dram_tensor("ar_out", [1, 8], F32, addr_space="Shared")

    RGRP = [list(range(NCORES))]

    with tile.TileContext(nc) as tc:
        nc.gpsimd.load_library(_mlp_lib)
        with (
            tc.tile_pool(name="const", bufs=1) as cp,
            tc.tile_pool(name="sb", bufs=3) as sp,
            tc.tile_pool(name="gp", bufs=2) as gp,
            tc.tile_pool(name="pp", bufs=2, space="PSUM") as pp,
        ):
            meta_sb = cp.tile([P, 2 * nchunk + P], F32)
            nc.sync.dma_start(meta_sb[:], meta[:])
            fc_sb = cp.tile([D, 4 * D + 6 + D], F32)
            nc.sync.dma_start(fc_sb[:], fconst[:])
            bidx_sb = cp.tile([P, 3 * BJ], I32)
            nc.sync.dma_start(bidx_sb[:], bidx[:])

            iota_sb = meta_sb[:, 2 * nchunk:2 * nchunk + P]
            w_g = [fc_sb[:, 0:D], fc_sb[:, 2 * D:3 * D]]
            w_m = [fc_sb[:, D:2 * D], fc_sb[:, 3 * D:4 * D]]
            bg08 = [fc_sb[:, 4 * D + 0:4 * D + 1], fc_sb[:, 4 * D + 3:4 * D + 4]]
            bg02 = [fc_sb[:, 4 * D + 1:4 * D + 2], fc_sb[:, 4 * D + 4:4 * D + 5]]
            bm = [fc_sb[:, 4 * D + 2:4 * D + 3], fc_sb[:, 4 * D + 5:4 * D + 6]]
            ident = fc_sb[:, 4 * D + 6:4 * D + 6 + D]

            ego_nat = cp.tile([P, NBLK, D], F32)

            scratch = pp.tile([1, 1], F32, tag="scr", bufs=1)
            nc.tensor.matmul(scratch[:], lhsT=fc_sb[:, 0:1], rhs=fc_sb[:, 0:1],
                             start=True, stop=True)

            # ---- BPR gathers + per-layer stats ----------------------------
            gb = {}
            ss = {}
            dp = {}
            dn = {}

            def bpr_layer(l, table):
                """Gather u/p/n rows of `table` for this core's 1024 slots and
                compute per-slot norms (Square accum) and u.p / u.n dots."""
                for role in range(3):
                    g = cp.tile([P, BJ, D], F32, name=f"gb{l}_{role}")
                    for j in range(BJ):
                        nc.gpsimd.indirect_dma_start(
                            out=g[:, j, :], out_offset=None, in_=table[:],
                            in_offset=bass.IndirectOffsetOnAxis(
                                ap=bidx_sb[:, role * BJ + j:role * BJ + j + 1],
                                axis=0))
                    gb[(l, role)] = g
                for role in range(3):
                    s = cp.tile([P, BJ], F32, name=f"ss{l}_{role}")
                    for j in range(BJ):
                        sq = sp.tile([P, D], F32, tag="sqscr")
                        nc.scalar.activation(sq[:], gb[(l, role)][:, j, :],
                                             AF.Square, accum_out=s[:, j:j + 1])
                    ss[(l, role)] = s
                for role, dst in ((1, dp), (2, dn)):
                    d = cp.tile([P, BJ], F32, name=f"d{l}_{role}")
                    for j in range(BJ):
                        m = sp.tile([P, D], F32, tag="dotscr")
                        nc.vector.tensor_tensor(m[:], gb[(l, 0)][:, j, :],
                                                gb[(l, role)][:, j, :], ALU.mult)
                        nc.vector.tensor_reduce(d[:, j:j + 1], m[:],
                                                mybir.AxisListType.X, ALU.add)
                    dst[l] = d

            bpr_layer(0, ego0)

            # ---- propagation layers --------------------------------------
            for l in range(2):
                table = ego0 if l == 0 else ego_full[0]
                for wi, blocks in enumerate(windows):
                    wstart, nw, parts = win_info[wi]
                    idx_w = sp.tile([P, max_nw * 8], I16, tag="idxw")
                    nc.sync.dma_start(idx_w[:, :nw * 8],
                                      idx_all[:, wstart * 8:(wstart + nw) * 8])
                    G = gp.tile([P, max_nw, D], F32, tag="G")
                    for ri, off, n_r in parts:
                        lo = ri * RANGE_ROWS
                        hi = min(NP_, lo + RANGE_ROWS)
                        # >=4096 idx per dma_gather crashes the device; cap at
                        # 16 chunks (2048 idx) per call, multi-packet.
                        for s in range(0, n_r, 16):
                            n_s = min(16, n_r - s)
                            o = off + s
                            nc.gpsimd.dma_gather(
                                out_ap=G[:, o:o + n_s, :],
                                in_ap=table[lo:hi, :],
                                idxs_ap=idx_w[:, o * 8:(o + n_s) * 8],
                                num_idxs=n_s * P, num_idxs_reg=n_s * P,
                                elem_size=D, single_packet=False,
                            )
                    # PE touch: absorb the gather waits once per window
                    nc.tensor.matmul(scratch[:], lhsT=G[:, 0, :1],
                                     rhs=G[:, 0, :1], start=True, stop=True)
                    for b in blocks:
                        cols_b = block_cols[b]
                        psum_side = pp.tile([D, P], F32, tag="side")
                        nchunks_b = len(cols_b)
                        for ci, col in enumerate(cols_b):
                            M = sp.tile([P, P], F32, tag="M", bufs=4)
                            nc.vector.tensor_scalar(
                                M[:], iota_sb,
                                meta_sb[:, col:col + 1],
                                meta_sb[:, nchunk + col:nchunk + col + 1],
                                ALU.is_equal, ALU.mult)
                            nc.tensor.matmul(
                                psum_side[:], lhsT=G[:, col - wstart, :], rhs=M[:],
                                start=(ci == 0), stop=(ci == nchunks_b - 1))
                        sideT = sp.tile([D, P], F32, tag="sideT")
                        nc.scalar.copy(sideT[:], psum_side[:])
                        p1 = pp.tile([D, P], F32, tag="dense")
                        nc.tensor.matmul(p1[:], lhsT=w_g[l], rhs=sideT[:],
                                         start=True, stop=True)
                        relu8 = sp.tile([D, P], F32, tag="relu8")
                        nc.scalar.activation(relu8[:], p1[:], AF.Relu,
                                             bias=bg08[l], scale=0.8)
                        uu = sp.tile([D, P], F32, tag="uu")
                        nc.vector.tensor_scalar(uu[:], p1[:], 0.2, bg02[l],
                                                ALU.mult, ALU.add)
                        gcnT = sp.tile([D, P], F32, tag="gcnT")
                        nc.vector.tensor_tensor(gcnT[:], uu[:], relu8[:], ALU.add)
                        p2 = pp.tile([D, P], F32, tag="dense")
                        nc.tensor.matmul(p2[:], lhsT=w_m[l], rhs=gcnT[:],
                                         start=True, stop=True)
                        egoT = sp.tile([D, P], F32, tag="egoT")
                        nc.scalar.activation(egoT[:], p2[:], AF.Identity,
                                             bias=bm[l])
                        p3 = pp.tile([P, D], F32, tag="p3")
                        nc.tensor.transpose(p3[:], egoT[:], ident)
                        nc.scalar.copy(ego_nat[:, b, :], p3[:])

                nc.sync.dma_start(
                    ego_blk[l][:].rearrange("(p r) d -> p (r d)", p=P),
                    ego_nat[:].rearrange("p r d -> p (r d)"))
                nc.gpsimd.collective_compute(
                    "AllGather", ALU.bypass, replica_groups=RGRP,
                    ins=[ego_blk[l][:]], outs=[ego_full[l][:]])
                bpr_layer(l + 1, ego_full[l])

            # ---- final combine -------------------------------------------
            def norm_term(d, su, so):
                t = sp.tile([P, BJ], F32, tag="nt", bufs=6)
                nc.vector.tensor_tensor(t[:], su[:], so[:], ALU.mult)
                t2 = sp.tile([P, BJ], F32, tag="nt", bufs=6)
                nc.scalar.activation(t2[:], t[:], AF.Sqrt)
                t3 = sp.tile([P, BJ], F32, tag="nt", bufs=6)
                nc.vector.reciprocal(t3[:], t2[:])
                t4 = sp.tile([P, BJ], F32, tag="nt", bufs=6)
                nc.vector.tensor_tensor(t4[:], d[:], t3[:], ALU.mult)
                return t4

            pos_s = cp.tile([P, BJ], F32)
            nc.vector.tensor_tensor(pos_s[:], dp[0][:],
                                    norm_term(dp[1], ss[(1, 0)], ss[(1, 1)])[:],
                                    ALU.add)
            nc.vector.tensor_tensor(pos_s[:], pos_s[:],
                                    norm_term(dp[2], ss[(2, 0)], ss[(2, 1)])[:],
                                    ALU.add)
            neg_s = cp.tile([P, BJ], F32)
            nc.vector.tensor_tensor(neg_s[:], dn[0][:],
                                    norm_term(dn[1], ss[(1, 0)], ss[(1, 2)])[:],
                                    ALU.add)
            nc.vector.tensor_tensor(neg_s[:], neg_s[:],
                                    norm_term(dn[2], ss[(2, 0)], ss[(2, 2)])[:],
                                    ALU.add)
            xdiff = cp.tile([P, BJ], F32)
            nc.vector.tensor_tensor(xdiff[:], neg_s[:], pos_s[:], ALU.subtract)
            ex = cp.tile([P, BJ], F32)
            nc.scalar.activation(ex[:], xdiff[:], AF.Exp)
            sp_ = cp.tile([P, BJ], F32)
            nc.scalar.activation(sp_[:], ex[:], AF.Ln, bias=1.0)

            reg_row = cp.tile([P, BJ], F32)
            nc.vector.tensor_tensor(reg_row[:], ss[(0, 0)][:], ss[(0, 1)][:],
                                    ALU.add)
            nc.vector.tensor_tensor(reg_row[:], reg_row[:], ss[(0, 2)][:],
                                    ALU.add)

            sc = cp.tile([P, 2], F32)
            srow = cp.tile([P, 1], F32)
            nc.vector.tensor_reduce(srow[:], sp_[:], mybir.AxisListType.X, ALU.add)
            nc.scalar.activation(sc[:, 0:1], srow[:], AF.Copy, scale=1.0 / B)
            rrow = cp.tile([P, 1], F32)
            nc.vector.tensor_reduce(rrow[:], reg_row[:], mybir.AxisListType.X,
                                    ALU.add)
            nc.scalar.activation(sc[:, 1:2], rrow[:], AF.Copy,
                                 scale=REG_LAMBDA * 0.5 / B)
            ones = cp.tile([P, 1], F32)
            nc.vector.memset(ones[:], 1.0)
            tot = pp.tile([1, 2], F32, tag="tot", bufs=1)
            nc.tensor.matmul(tot[:], lhsT=ones[:], rhs=sc[:], start=True,
                             stop=True)
            ar_sb = cp.tile([1, 8], F32)
            nc.vector.memset(ar_sb[:], 0.0)
            nc.scalar.copy(ar_sb[:, 0:2], tot[:])
            nc.sync.dma_start(ar_in[:], ar_sb[:])
            nc.gpsimd.collective_compute(
                "AllReduce", ALU.add, replica_groups=RGRP,
                ins=[ar_in[:]], outs=[ar_out[:]])
            nc.sync.dma_start(out_ext[:], ar_out[:1, 0:2])
    nc.compile()
    return nc


def run(inputs, trace=False, trace_cores=None):
    inputs = {k: np.asarray(v) for k, v in inputs.items()}
    sched, in_maps = prep(inputs)
    nc = build(sched)
    kw = {}
    if trace:
        kw = dict(trace=True, trace_cores=trace_cores or list(range(NCORES)))
    res = run_bass_kernel_spmd(nc, in_maps, list(range(NCORES)), **kw)
    out = res.results[0]["out"].reshape(2).astype(np.float32)
    return out, res


def kernel(**inputs):
    out, _ = run(inputs)
    return out
